# revision 11
# baseline (speedup 1.0000x reference)
"""LUKE entity-aware attention block on 8 Trainium2 NeuronCores.

Data-parallel over batch: B=16 -> 2 batches per core. Each core runs the
full block (4-way Q projections, shared K/V over the 640-token concat
sequence, softmax attention, output proj + LN, FFN + LN) on its 2 batches.

Layout strategy per core/batch:
  - activations feature-major [D-part, T-free] for projections (rhs),
  - scores computed transposed S^T[k, q] = K_fm^T @ Q_fm so softmax sums
    arrive via an extra ones-column in V (PV matmul computes ctx and the
    softmax denominators in one pass); no max-subtraction (scores are O(1)),
  - context comes out feature-major so AO proj needs no transpose; kv_in
    and ao are PE-transposed via identity matmuls,
  - LayerNorms token-major with bn_stats/bn_aggr.
Matmul operands bf16 (fp32 accumulate in PSUM); everything else fp32.
W_o is staged to a DRAM bf16 copy and streamed per t-group (SBUF pressure).
"""

import numpy as np
from contextlib import ExitStack

B, S, E, T, D, H, DH, FF = 16, 512, 128, 640, 768, 12, 64, 3072
NCORES = 8
BPC = B // NCORES           # batches per core
TT = T // 128               # 5 token tiles (0..3 word, 4 entity)
DT = D // 128               # 6 feature tiles
FT = FF // 128              # 24 ff tiles
EPS = 1e-12

_CACHE = {}


def _build():
    import concourse.bass as bass
    import concourse.mybir as mybir
    import concourse.tile as tile
    from concourse import bacc
    from concourse.masks import make_identity

    F32 = mybir.dt.float32
    BF16 = mybir.dt.bfloat16
    AF = mybir.ActivationFunctionType
    ALU = mybir.AluOpType

    nc = bacc.Bacc("TRN2", target_bir_lowering=False)

    din = {}
    def dt_in(name, shape):
        din[name] = nc.dram_tensor(name, shape, F32, kind="ExternalInput")
        return din[name]

    wh = dt_in("wh", (BPC, S, D))
    eh = dt_in("eh", (BPC, E, D))
    msk = dt_in("msk", (BPC, T))
    for nm in ("w_q", "w_k", "w_v", "w_w2e", "w_e2w", "w_e2e", "w_ao"):
        dt_in(nm, (D, D))
    dt_in("w_i", (D, FF))
    dt_in("w_o", (FF, D))
    for nm in ("b_q", "b_k", "b_v", "b_w2e", "b_e2w", "b_e2e", "b_ao",
               "g_ao", "beta_ao", "b_o", "g_o", "beta_o"):
        dt_in(nm, (D,))
    dt_in("b_i", (FF,))
    out_w = nc.dram_tensor("out_w", (BPC, S, D), F32, kind="ExternalOutput")
    out_e = nc.dram_tensor("out_e", (BPC, E, D), F32, kind="ExternalOutput")

    with tile.TileContext(nc) as tc, ExitStack() as es:
        # ---------- pools ----------
        wts = es.enter_context(tc.tile_pool(name="wts", bufs=1))
        vecs = es.enter_context(tc.tile_pool(name="vecs", bufs=1))
        ps = es.enter_context(tc.tile_pool(name="ps", bufs=4, space="PSUM"))
        dramp = es.enter_context(tc.tile_pool(name="dramp", bufs=1, space="DRAM"))

        cast_engs = [
            lambda out, in_: nc.vector.tensor_copy(out=out, in_=in_),
            lambda out, in_: nc.scalar.copy(out=out, in_=in_),
            lambda out, in_: nc.gpsimd.tensor_copy(out=out, in_=in_),
        ]
        n_cast = 0

        # bf16 resident weights (w_o goes to a DRAM bf16 copy instead)
        wbf = {}
        wo_dram = dramp.tile([FT, 128, D], BF16, name="wo_dram")
        with tc.tile_pool(name="wstage", bufs=3) as stage:
            def stagecast(dst_ap, src_ap):
                nonlocal n_cast
                st = stage.tile(list(src_ap.shape), F32, tag="wstage", name="wst")
                nc.sync.dma_start(out=st, in_=src_ap)
                cast_engs[n_cast % 3](dst_ap, st)
                n_cast += 1

            for nm in ("w_q", "w_k", "w_v", "w_w2e", "w_e2w", "w_e2e", "w_ao"):
                wbf[nm] = []
                for di in range(DT):
                    t = wts.tile([128, D], BF16, tag=f"{nm}_{di}", name=f"{nm}_{di}")
                    stagecast(t, din[nm][di * 128:(di + 1) * 128, :])
                    wbf[nm].append(t)
            wbf["w_i"] = []
            for di in range(DT):
                t = wts.tile([128, FF], BF16, tag=f"w_i_{di}", name=f"w_i_{di}")
                for half in range(2):
                    stagecast(t[:, half * 1536:(half + 1) * 1536],
                              din["w_i"][di * 128:(di + 1) * 128,
                                         half * 1536:(half + 1) * 1536])
                wbf["w_i"].append(t)
            for fi in range(FT):
                t = stage.tile([128, D], BF16, tag="wobf", name="wobf")
                stagecast(t, din["w_o"][fi * 128:(fi + 1) * 128, :])
                nc.sync.dma_start(out=wo_dram[fi], in_=t)

        # small vectors: feature-major biases [128, n] and single rows [1, D]
        bfm = {}
        for nm in ("b_q", "b_k", "b_w2e", "b_e2w", "b_e2e"):
            t = vecs.tile([128, DT], F32, tag=f"{nm}_fm", name=f"{nm}_fm")
            nc.sync.dma_start(out=t, in_=din[nm].rearrange("(n p) -> p n", p=128))
            bfm[nm] = t
        b_i_fm = vecs.tile([128, FT], F32, tag="b_i_fm")
        nc.sync.dma_start(out=b_i_fm, in_=din["b_i"].rearrange("(n p) -> p n", p=128))
        rows = {}
        for nm in ("b_v", "b_ao", "g_ao", "beta_ao", "b_o", "g_o", "beta_o"):
            t = vecs.tile([1, D], F32, tag=f"{nm}_row", name=f"{nm}_row")
            nc.sync.dma_start(out=t, in_=din[nm][None, :])
            rows[nm] = t

        eps_t = vecs.tile([128, 1], F32, tag="eps")
        nc.vector.memset(eps_t, EPS)
        ident = vecs.tile([128, 128], F32, tag="ident")
        make_identity(nc, ident)

        # ---------- per-batch pools ----------
        iop = es.enter_context(tc.tile_pool(name="iop", bufs=1))     # aog
        actp = es.enter_context(tc.tile_pool(name="actp", bufs=1))   # fm acts
        qkp = es.enter_context(tc.tile_pool(name="qkp", bufs=1))     # q/k tiles
        expp = es.enter_context(tc.tile_pool(name="expp", bufs=5))   # exp(S^T)
        smallp = es.enter_context(tc.tile_pool(name="smallp", bufs=4))
        scr = es.enter_context(tc.tile_pool(name="scr", bufs=8))     # [128,D] f32
        trp = es.enter_context(tc.tile_pool(name="trp", bufs=2))     # misc bf16
        wop = es.enter_context(tc.tile_pool(name="wop", bufs=2))     # w_o stream

        def scrt(name):
            return scr.tile([128, D], F32, tag="scratch", name=name)

        def bc(nm, name):
            t = scrt(name)
            nc.gpsimd.partition_broadcast(t, rows[nm][0:1, :])
            return t

        def ln_apply(pre, g_bc, beta_bc, out_t):
            """LayerNorm(pre) * g + beta -> out_t (token-major [128, D] f32)."""
            stats = smallp.tile([128, 3, 6], F32, tag="lnstats", name="lnstats")
            prer = pre.rearrange("p (n f) -> p n f", f=256)
            for i in range(3):
                nc.vector.bn_stats(out=stats[:, i, :], in_=prer[:, i, :])
            mv = smallp.tile([128, 2], F32, tag="lnmv", name="lnmv")
            nc.vector.bn_aggr(out=mv, in_=stats)
            std = smallp.tile([128, 1], F32, tag="lnstd", name="lnstd")
            nc.scalar.activation(out=std, in_=mv[:, 1:2], func=AF.Sqrt, bias=eps_t)
            rstd = smallp.tile([128, 1], F32, tag="lnrstd", name="lnrstd")
            nc.vector.reciprocal(out=rstd, in_=std)
            normed = scrt("lnnorm")
            nc.vector.tensor_scalar(out=normed, in0=pre, scalar1=mv[:, 0:1],
                                    scalar2=rstd, op0=ALU.subtract, op1=ALU.mult)
            tmp = scrt("lntmp")
            nc.gpsimd.tensor_mul(out=tmp, in0=normed, in1=g_bc)
            nc.vector.tensor_add(out=out_t, in0=tmp, in1=beta_bc)

        def load_kv_row(b, t, name):
            kt = scrt(name)
            if t < 4:
                nc.sync.dma_start(out=kt, in_=wh[b, t * 128:(t + 1) * 128, :])
            else:
                nc.sync.dma_start(out=kt, in_=eh[b, :, :])
            return kt

        for b in range(BPC):
            mask_t = []
            for t in range(TT):
                mt = smallp.tile([128, 1], F32, tag=f"mask{t}", name=f"mask{t}")
                nc.sync.dma_start(out=mt, in_=msk[b, t * 128:(t + 1) * 128][:, None])
                mask_t.append(mt)

            # ---- kv feature-major (PE transpose f32 -> bf16) ----
            kv_fm = []
            for dj in range(DT):
                kv_fm.append(actp.tile([128, T], BF16, tag=f"fm{dj}", name=f"kvfm{dj}"))
            for t in range(TT):
                kt = load_kv_row(b, t, f"kvin{t}")
                for dj in range(DT):
                    pt = ps.tile([128, 128], F32, tag="ps", name="trps")
                    nc.tensor.transpose(out=pt, in_=kt[:, dj * 128:(dj + 1) * 128],
                                        identity=ident)
                    nc.scalar.copy(out=kv_fm[dj][:, t * 128:(t + 1) * 128], in_=pt)

            # ---- V1 token-major [128, H*65] (64 v cols + ones col per head) ----
            bv_bc = bc("b_v", "bvbc")
            v1 = []
            for t in range(TT):
                vt = actp.tile([128, H * 65], BF16, tag=f"v1_{t}", name=f"v1_{t}")
                ones_ap = vt.rearrange("p (h c) -> p h c", c=65)[:, :, 64]
                nc.vector.memset(ones_ap, 1.0)
                vp = ps.tile([128, D], F32, tag="ps", name="vps")
                for (d0, dw) in ((0, 512), (512, 256)):
                    for di in range(DT):
                        nc.tensor.matmul(vp[:, d0:d0 + dw],
                                         lhsT=kv_fm[di][:, t * 128:(t + 1) * 128],
                                         rhs=wbf["w_v"][di][:, d0:d0 + dw],
                                         start=(di == 0), stop=(di == DT - 1))
                for h in range(H):
                    nc.vector.tensor_add(out=vt[:, h * 65:h * 65 + 64],
                                         in0=vp[:, h * 64:(h + 1) * 64],
                                         in1=bv_bc[:, h * 64:(h + 1) * 64])
                v1.append(vt)

            # ---- attention, head-pair (= d'-tile) at a time ----
            ctx_bf = []
            for dj in range(DT):
                ctx_bf.append(actp.tile([128, T], BF16, tag=f"ctx{dj}", name=f"ctx{dj}"))

            for dj in range(DT):
                cols = slice(dj * 128, (dj + 1) * 128)
                qw_t = qkp.tile([128, T], BF16, tag="qw", name="qw")
                qe_t = qkp.tile([128, T], BF16, tag="qe", name="qe")
                k_t = qkp.tile([128, T], BF16, tag="kk", name="kk")
                for dst, wword, went, bword, bent in (
                        (qw_t, "w_q", "w_e2w", "b_q", "b_e2w"),
                        (qe_t, "w_w2e", "w_e2e", "b_w2e", "b_e2e"),
                        (k_t, "w_k", "w_k", "b_k", "b_k")):
                    pp = ps.tile([128, T], F32, tag="ps", name="qkps")
                    for di in range(DT):
                        nc.tensor.matmul(pp[:, 0:512], lhsT=wbf[wword][di][:, cols],
                                         rhs=kv_fm[di][:, 0:512],
                                         start=(di == 0), stop=(di == DT - 1))
                    for di in range(DT):
                        nc.tensor.matmul(pp[:, 512:640], lhsT=wbf[went][di][:, cols],
                                         rhs=kv_fm[di][:, 512:640],
                                         start=(di == 0), stop=(di == DT - 1))
                    nc.vector.tensor_scalar_add(out=dst[:, 0:512], in0=pp[:, 0:512],
                                                scalar1=bfm[bword][:, dj:dj + 1])
                    nc.vector.tensor_scalar_add(out=dst[:, 512:640], in0=pp[:, 512:640],
                                                scalar1=bfm[bent][:, dj:dj + 1])

                for hh in range(2):
                    h = 2 * dj + hh
                    hrows = slice(hh * 64, (hh + 1) * 64)
                    cp = ps.tile([65, T], F32, tag="ps", name="ctxps")
                    for kt in range(TT):
                        qsrc = qw_t if kt < 4 else qe_t
                        sp = ps.tile([128, T], F32, tag="ps", name="scps")
                        for (q0, qw_) in ((0, 512), (512, 128)):
                            nc.tensor.matmul(sp[:, q0:q0 + qw_],
                                             lhsT=k_t[hrows, kt * 128:(kt + 1) * 128],
                                             rhs=qsrc[hrows, q0:q0 + qw_],
                                             start=True, stop=True)
                        et = expp.tile([128, T], BF16, tag="expst", name="expst")
                        nc.scalar.activation(out=et, in_=sp, func=AF.Exp,
                                             scale=float(1.0 / np.sqrt(DH)),
                                             bias=mask_t[kt])
                        for (q0, qw_) in ((0, 512), (512, 128)):
                            nc.tensor.matmul(cp[:, q0:q0 + qw_],
                                             lhsT=v1[kt][:, h * 65:h * 65 + 65],
                                             rhs=et[:, q0:q0 + qw_],
                                             start=(kt == 0), stop=(kt == TT - 1))
                    rsum = smallp.tile([1, T], F32, tag="rsum", name="rsum", bufs=2)
                    nc.vector.reciprocal(out=rsum, in_=cp[64:65, :])
                    rsb = scr.tile([64, T], F32, tag="scratch", name="rsb")
                    nc.gpsimd.partition_broadcast(rsb, rsum[0:1, :])
                    nc.vector.tensor_mul(out=ctx_bf[dj][hrows, :], in0=cp[0:64, :],
                                         in1=rsb)

            # ---- attention output proj + residual + LN1 ----
            bao_bc = bc("b_ao", "baobc")
            gao_bc = bc("g_ao", "gaobc")
            betaao_bc = bc("beta_ao", "betaaobc")
            aog = []
            for t in range(TT):
                ap_ = ps.tile([128, D], F32, tag="ps", name="aops")
                for (d0, dw) in ((0, 512), (512, 256)):
                    for di in range(DT):
                        nc.tensor.matmul(ap_[:, d0:d0 + dw],
                                         lhsT=ctx_bf[di][:, t * 128:(t + 1) * 128],
                                         rhs=wbf["w_ao"][di][:, d0:d0 + dw],
                                         start=(di == 0), stop=(di == DT - 1))
                kvt = load_kv_row(b, t, f"kvres{t}")
                kvb = scrt("kvb")
                nc.gpsimd.tensor_add(out=kvb, in0=kvt, in1=bao_bc)
                pre = scrt("aopre")
                nc.vector.tensor_add(out=pre, in0=ap_, in1=kvb)
                at = iop.tile([128, D], F32, tag=f"aog{t}", name=f"aog{t}")
                ln_apply(pre, gao_bc, betaao_bc, at)
                aog.append(at)

            # ---- transpose aog -> feature-major bf16 (reuses fm tags) ----
            ao_fm = []
            for dj in range(DT):
                ao_fm.append(actp.tile([128, T], BF16, tag=f"fm{dj}", name=f"aofm{dj}"))
            for t in range(TT):
                for dj in range(DT):
                    pt = ps.tile([128, 128], F32, tag="ps", name="trps2")
                    nc.tensor.transpose(out=pt, in_=aog[t][:, dj * 128:(dj + 1) * 128],
                                        identity=ident)
                    nc.scalar.copy(out=ao_fm[dj][:, t * 128:(t + 1) * 128], in_=pt)

            # ---- FFN + residual + LN2, t-groups of 3 and 2 token tiles ----
            bo_bc = bc("b_o", "bobc")
            go_bc = bc("g_o", "gobc")
            betao_bc = bc("beta_o", "betaobc")
            for (t0, ntt) in ((0, 3), (3, 2)):
                tgs = slice(t0 * 128, (t0 + ntt) * 128)
                tgw = ntt * 128
                outp = []
                for j in range(ntt):
                    outp.append(ps.tile([128, D], F32, tag="ps", name=f"outp{j}"))
                for fi in range(FT):
                    wo_t = wop.tile([128, D], BF16, tag="wo", name="wo_t")
                    nc.sync.dma_start(out=wo_t, in_=wo_dram[fi])
                    ip = ps.tile([128, tgw], F32, tag="ps", name="interps")
                    for di in range(DT):
                        nc.tensor.matmul(ip, lhsT=wbf["w_i"][di][:, fi * 128:(fi + 1) * 128],
                                         rhs=ao_fm[di][:, tgs],
                                         start=(di == 0), stop=(di == DT - 1))
                    ib = trp.tile([128, 384], BF16, tag="interbf", name="interbf")
                    nc.scalar.activation(out=ib[:, 0:tgw], in_=ip, func=AF.Gelu,
                                         bias=b_i_fm[:, fi:fi + 1])
                    for j in range(ntt):
                        for (d0, dw) in ((0, 512), (512, 256)):
                            nc.tensor.matmul(outp[j][:, d0:d0 + dw],
                                             lhsT=ib[:, j * 128:(j + 1) * 128],
                                             rhs=wo_t[:, d0:d0 + dw],
                                             start=(fi == 0), stop=(fi == FT - 1))
                for j in range(ntt):
                    t = t0 + j
                    y1 = scrt("y1")
                    nc.vector.tensor_add(out=y1, in0=outp[j], in1=aog[t])
                    y2 = scrt("y2")
                    nc.gpsimd.tensor_add(out=y2, in0=y1, in1=bo_bc)
                    fin = scrt("fin")
                    ln_apply(y2, go_bc, betao_bc, fin)
                    if t < 4:
                        nc.sync.dma_start(out=out_w[b, t * 128:(t + 1) * 128, :], in_=fin)
                    else:
                        nc.sync.dma_start(out=out_e[b, :, :], in_=fin)

    nc.compile()
    return nc


def _in_maps(inputs):
    f32c = lambda a: np.ascontiguousarray(np.asarray(a), dtype=np.float32)
    word = f32c(inputs["word_hidden_states"])
    ent = f32c(inputs["entity_hidden_states"])
    mask = f32c(inputs["attention_mask"]).reshape(B, T)
    wmap = {
        "w_q": "W_q", "w_k": "W_k", "w_v": "W_v", "w_w2e": "W_w2e",
        "w_e2w": "W_e2w", "w_e2e": "W_e2e", "w_ao": "W_ao", "w_i": "W_i",
        "w_o": "W_o", "b_q": "b_q", "b_k": "b_k", "b_v": "b_v",
        "b_w2e": "b_w2e", "b_e2w": "b_e2w", "b_e2e": "b_e2e", "b_ao": "b_ao",
        "g_ao": "g_ao", "beta_ao": "beta_ao", "b_i": "b_i", "b_o": "b_o",
        "g_o": "g_o", "beta_o": "beta_o",
    }
    shared = {k: f32c(inputs[v]) for k, v in wmap.items()}
    maps = []
    for c in range(NCORES):
        sl = slice(c * BPC, (c + 1) * BPC)
        m = dict(shared)
        m["wh"] = word[sl]
        m["eh"] = ent[sl]
        m["msk"] = mask[sl]
        maps.append(m)
    return maps


def kernel(**inputs):
    from concourse.bass_utils import run_bass_kernel_spmd

    if "nc" not in _CACHE:
        _CACHE["nc"] = _build()
    nc = _CACHE["nc"]
    res = run_bass_kernel_spmd(nc, _in_maps(inputs), core_ids=list(range(NCORES)))
    word = np.concatenate([r["out_w"] for r in res.results], axis=0)
    ent = np.concatenate([r["out_e"] for r in res.results], axis=0)
    return word, ent
